# revision 1
# baseline (speedup 1.0000x reference)
"""Trainium2 Bass kernel for nn_CNN2D_37873021616665 (dense_cnn).

Data-parallel over batch: 16 samples -> 8 cores x 2 samples. Each core:
  1. streams its x shard [2,64,224,224] through a 32x32 mean-pool done on the
     PE (block-diagonal 1/1024 matrices as stationary operand, h contracted on
     partitions, w binned by PSUM accumulation over 32 column offsets)
  2. runs the attention head (1x1 conv, 3x3 conv, the torch-.view quirky
     [B,196]->[49,4] regroup via a tiny DRAM round-trip, LN over 4, softmax
     over 49, segment sums) and ROI head (3x3 conv, spatial mean, linear)
  3. roi_align via dynamic row-pair gather DMAs (register offsets) and a
     separable bilinear combine with clamp/validity folded into weights
  4. final per-sample 3x3 conv by linearity: conv with the 4 base kernels,
     combined with the attention-derived scalars aw.
"""
import numpy as np

B = 16
NCORES = 8
BL = B // NCORES          # samples per core
C = 64
H = W = 224
HW = H * W
POOL = 7
NK = 4
TEMP = 5.0

_CACHED = {}
BF16_POOL = True      # stream x through the mean-pool in bf16 (gathers stay fp32)


# --------------------------------------------------------------------------
# host-side constant packing (pure layout; replicated to all cores)
# --------------------------------------------------------------------------
def make_consts(inputs):
    f32 = np.float32
    fc1_w = np.asarray(inputs["fc1_w"], f32)
    fc2_w = np.asarray(inputs["fc2_w"], f32)
    fc2_b = np.asarray(inputs["fc2_b"], f32)
    ln_g = np.asarray(inputs["ln_g"], f32)
    ln_b = np.asarray(inputs["ln_b"], f32)
    roi_w1 = np.asarray(inputs["roi_w1"], f32)
    roi_w2 = np.asarray(inputs["roi_w2"], f32)
    roi_b2 = np.asarray(inputs["roi_b2"], f32)
    weight = np.asarray(inputs["weight"], f32)
    bias = np.asarray(inputs["bias"], f32)

    k128 = np.zeros((128, 7), f32)
    for h in range(128):
        k128[h, h // 32] = 1.0 / 1024.0
    k96 = np.zeros((96, 7), f32)
    for h in range(96):
        k96[h, 4 + h // 32] = 1.0 / 1024.0
    if BF16_POOL:
        import ml_dtypes
        k128 = k128.astype(ml_dtypes.bfloat16)   # 1/1024 is exact in bf16
        k96 = k96.astype(ml_dtypes.bfloat16)

    k64 = np.zeros((64, 2468), f32)
    k64[:, 0:16] = fc1_w[:, :, 0, 0].T
    k64[:, 16:160] = roi_w1.transpose(1, 2, 3, 0).reshape(64, 144)
    k64[:, 160:2464] = weight.transpose(2, 0, 3, 4, 1).reshape(64, 2304)
    k64[:, 2464:2468] = bias.T

    k49 = np.zeros((49, 33), f32)
    k49[:, 0:8] = np.tile(ln_g, (49, 2))
    k49[:, 8:16] = np.tile(ln_b, (49, 2))
    t = np.arange(196).reshape(49, 4)
    for s in range(4):
        k49[:, 16 + s * 4:16 + (s + 1) * 4] = ((t >= 49 * s) & (t < 49 * (s + 1)))
    k49[:, 32] = 1.0

    k16 = np.zeros((16, 40), f32)
    k16[:, 0:36] = fc2_w.transpose(1, 2, 3, 0).reshape(16, 36)
    k16[:, 36:40] = (roi_w2[:4, :, 0, 0] / 49.0).T

    k7 = np.eye(7, dtype=f32)

    k4 = np.zeros((4, 2), f32)
    k4[:, 0] = fc2_b
    k4[:, 1] = roi_b2[:4] * 224.0

    k1 = np.zeros((1, 78), f32)
    off = (np.arange(7, dtype=f32)[:, None] + (np.arange(2, dtype=f32)[None, :] + 0.5) / 2)
    k1[0, 0:14] = off.reshape(-1)
    k1[0, 14:78] = 1.0

    return {"k128": k128, "k96": k96, "k64": k64, "k49": k49,
            "k16": k16, "k7": k7, "k4": k4, "k1": k1}


# --------------------------------------------------------------------------
# device program
# --------------------------------------------------------------------------
def build_nc(debug_out=False, repeat=1):
    import concourse.bass as bass
    import concourse.bacc as bacc
    import concourse.mybir as mybir
    import concourse.tile as tile
    from concourse.bass import ds

    f32 = mybir.dt.float32
    i32 = mybir.dt.int32
    bf16 = mybir.dt.bfloat16
    pdt = bf16 if BF16_POOL else f32
    Alu = mybir.AluOpType
    Act = mybir.ActivationFunctionType

    nc = bacc.Bacc("TRN2", target_bir_lowering=False)

    xs_h = nc.dram_tensor("xs", [BL, C, H, W], f32, kind="ExternalInput")
    # xb is host-pretransposed to [b, h, c, w]: the pooling loads then read one
    # contiguous (c, w) slab per h-partition -> long DMA runs at line rate
    xb_h = nc.dram_tensor("xb", [BL, H, C, W], bf16 if BF16_POOL else f32,
                          kind="ExternalInput")
    k128_h = nc.dram_tensor("k128", [128, 7], pdt, kind="ExternalInput")
    k96_h = nc.dram_tensor("k96", [96, 7], pdt, kind="ExternalInput")
    k64_h = nc.dram_tensor("k64", [64, 2468], f32, kind="ExternalInput")
    k49_h = nc.dram_tensor("k49", [49, 33], f32, kind="ExternalInput")
    k16_h = nc.dram_tensor("k16", [16, 40], f32, kind="ExternalInput")
    k7_h = nc.dram_tensor("k7", [7, 7], f32, kind="ExternalInput")
    k4_h = nc.dram_tensor("k4", [4, 2], f32, kind="ExternalInput")
    k1_h = nc.dram_tensor("k1", [1, 78], f32, kind="ExternalInput")
    out_h = nc.dram_tensor("out", [BL, C, POOL, POOL], f32, kind="ExternalOutput")
    if debug_out:
        dbg_ctxT = nc.dram_tensor("dbg_ctxT", [BL, 64, 49], f32, kind="ExternalOutput")
        dbg_v4 = nc.dram_tensor("dbg_v4", [49, BL, 4], f32, kind="ExternalOutput")
        dbg_aw = nc.dram_tensor("dbg_aw", [1, BL * 4], f32, kind="ExternalOutput")
        dbg_bx = nc.dram_tensor("dbg_bx", [BL, 1, 4], f32, kind="ExternalOutput")
        dbg_io = nc.dram_tensor("dbg_io", [BL, 1, 2, 14], i32, kind="ExternalOutput")
        dbg_wb = nc.dram_tensor("dbg_wb", [BL, 1, 56], f32, kind="ExternalOutput")
        dbg_zm = nc.dram_tensor("dbg_zm", [BL, 64, 9, 9], f32, kind="ExternalOutput")
        dbg_g2 = nc.dram_tensor("dbg_g2", [BL, 64, 448], f32, kind="ExternalOutput")

    xs_flat = xs_h[:].rearrange("b c h w -> b c (h w)")

    from contextlib import ExitStack

    with tile.TileContext(nc) as tc, ExitStack() as est:
        cp = est.enter_context(tc.tile_pool(name="const", bufs=1))
        # bufs=4: four (b, cg) iterations total -> no slot reuse, so the big
        # HWDGE loads never need sync waits (walrus DIRECT2D wait limit)
        xp = est.enter_context(tc.tile_pool(name="xtiles", bufs=4))
        wp = est.enter_context(tc.tile_pool(name="work", bufs=2))
        gp = est.enter_context(tc.tile_pool(name="gather", bufs=1))
        pp_acc = est.enter_context(tc.tile_pool(name="ps_acc", bufs=2, space="PSUM"))
        pp_m = est.enter_context(tc.tile_pool(name="ps_misc", bufs=2, space="PSUM"))
        pp_c = est.enter_context(tc.tile_pool(name="ps_conv", bufs=1, space="PSUM"))
        dp = est.enter_context(tc.tile_pool(name="dscratch", bufs=1, space="DRAM"))

        if True:
            # ---- load constants ----
            kc = {}
            for name, hdl, shp in (("k128", k128_h, [128, 7]), ("k96", k96_h, [96, 7]),
                                   ("k64", k64_h, [64, 2468]), ("k49", k49_h, [49, 33]),
                                   ("k16", k16_h, [16, 40]), ("k7", k7_h, [7, 7]),
                                   ("k4", k4_h, [4, 2]), ("k1", k1_h, [1, 78])):
                t = cp.tile(shp, pdt if name in ("k128", "k96") else f32, tag=name)
                nc.sync.dma_start(out=t[:], in_=hdl[:])
                kc[name] = t
            fc1l = kc["k64"][:, 0:16]
            roi1l = lambda d: kc["k64"][:, 16 + d * 16:16 + (d + 1) * 16]
            wl = lambda k, d: kc["k64"][:, 160 + k * 576 + d * 64:160 + k * 576 + (d + 1) * 64]
            biasT = kc["k64"][:, 2464:2468]
            lng8 = kc["k49"][:, 0:8]
            lnb8 = kc["k49"][:, 8:16]
            m4 = lambda s: kc["k49"][:, 16 + s * 4:16 + (s + 1) * 4]
            ones49 = kc["k49"][:, 32:33]
            fc2l = lambda d: kc["k16"][:, d * 4:(d + 1) * 4]
            w2l = kc["k16"][:, 36:40]
            id7 = kc["k7"][:]
            fc2b = kc["k4"][:, 0:1]
            b224 = kc["k4"][:, 1:2]
            offv = kc["k1"][:, 0:14]
            ones1x = kc["k1"][:, 14:78]          # [1, 64] of ones

            for _rep in range(repeat):
                attflat = dp.tile([BL, 196], f32)

                ctxT_l, ctx9_l, zoomed_l, awcol_l = [], [], [], []

                # ================= per-sample: pooling + heads =================
                for b in range(BL):
                    # ---- pooling ----
                    ps_pool = pp_acc.tile([7, 2, 32, 7], f32, tag="poolacc")
                    for cg in range(2):
                        ta = xp.tile([128, 32, 224], pdt, tag="TA")
                        tb = xp.tile([96, 32, 224], pdt, tag="TB")
                        # split the two loads across both HWDGE rings (SP + ACT)
                        nc.sync.dma_start(
                            out=ta[:], in_=xb_h[b, 0:128, cg * 32:(cg + 1) * 32, :])
                        nc.scalar.dma_start(
                            out=tb[:], in_=xb_h[b, 128:224, cg * 32:(cg + 1) * 32, :])
                        ta4 = ta[:].rearrange("p c (j t) -> p c j t", t=32)
                        tb4 = tb[:].rearrange("p c (j t) -> p c j t", t=32)
                        for s in range(32):
                            nc.tensor.matmul(ps_pool[:, cg], kc["k128"][:], ta4[:, :, :, s],
                                             start=(s == 0), stop=False)
                        for s in range(32):
                            nc.tensor.matmul(ps_pool[:, cg], kc["k96"][:], tb4[:, :, :, s],
                                             start=False, stop=(s == 31))

                    ctx_sb = wp.tile([7, 2, 32, 7], f32, tag="ctx_sb")
                    nc.vector.tensor_copy(ctx_sb[:], ps_pool[:])

                    # ---- transpose context to [c, i, j] ----
                    ps_ctxT = pp_m.tile([64, 7, 7], f32, tag="psm")
                    for j in range(7):
                        nc.tensor.transpose(ps_ctxT[:, :, j], ctx_sb[:, :, :, j], id7)
                    ctxT = wp.tile([64, 7, 7], f32, tag="ctxT")
                    nc.vector.tensor_copy(ctxT[:], ps_ctxT[:])
                    ctx9 = wp.tile([64, 9, 9], f32, tag="ctx9")
                    nc.gpsimd.memset(ctx9[:], 0.0)
                    nc.vector.tensor_copy(ctx9[:, 1:8, 1:8], ctxT[:])
                    ctxT_l.append(ctxT)
                    ctx9_l.append(ctx9)
                    if debug_out:
                        nc.gpsimd.dma_start(out=dbg_ctxT[b], in_=ctxT[:].rearrange("c i j -> c (i j)"))

                    # ---- attention head ----
                    ps_a = pp_m.tile([16, 7, 7], f32, tag="psm")
                    nc.tensor.matmul(ps_a[:], fc1l, ctxT[:], start=True, stop=True)
                    a_sb = wp.tile([16, 7, 7], f32, tag="a_sb")
                    nc.scalar.activation(a_sb[:], ps_a[:], Act.Relu)
                    a9 = wp.tile([16, 9, 9], f32, tag="a9")
                    nc.gpsimd.memset(a9[:], 0.0)
                    nc.vector.tensor_copy(a9[:, 1:8, 1:8], a_sb[:])
                    ps_att = pp_m.tile([4, 7, 7], f32, tag="psm")
                    for d in range(9):
                        dy, dx = d // 3, d % 3
                        nc.tensor.matmul(ps_att[:], fc2l(d), a9[:, dy:dy + 7, dx:dx + 7],
                                         start=(d == 0), stop=(d == 8))
                    att_sb = wp.tile([4, 7, 7], f32, tag="att_sb")
                    nc.scalar.activation(att_sb[:], ps_att[:], Act.Identity, bias=fc2b, scale=1.0)
                    nc.gpsimd.dma_start(
                        out=attflat[b].rearrange("(k q) -> k q", k=4), in_=att_sb[:].rearrange("k i j -> k (i j)"))

                # ================= regroup + LN + softmax + aw (both samples) =================
                v4 = wp.tile([49, BL, 4], f32, tag="v4")
                nc.gpsimd.dma_start(out=v4[:], in_=attflat[:].rearrange("b (p k) -> p b k", k=4))
                s1 = wp.tile([49, BL], f32, tag="s1")
                nc.vector.tensor_reduce(s1[:], v4[:], op=Alu.add, axis=mybir.AxisListType.X)
                sq = wp.tile([49, BL, 4], f32, tag="sq")
                nc.scalar.activation(sq[:], v4[:], Act.Square)
                s2 = wp.tile([49, BL], f32, tag="s2")
                nc.vector.tensor_reduce(s2[:], sq[:], op=Alu.add, axis=mybir.AxisListType.X)
                mu = wp.tile([49, BL], f32, tag="mu")
                nc.vector.tensor_scalar_mul(mu[:], s1[:], 0.25)
                mu2 = wp.tile([49, BL], f32, tag="mu2")
                nc.vector.tensor_mul(mu2[:], mu[:], mu[:])
                var = wp.tile([49, BL], f32, tag="var")
                nc.vector.scalar_tensor_tensor(var[:], s2[:], 0.25, mu2[:],
                                               op0=Alu.mult, op1=Alu.subtract)
                nc.vector.tensor_scalar_add(var[:], var[:], 1e-5)
                rec = wp.tile([49, BL], f32, tag="rec")
                nc.vector.reciprocal(rec[:], var[:])
                rstd = wp.tile([49, BL], f32, tag="rstd")
                nc.scalar.activation(rstd[:], rec[:], Act.Sqrt)
                y = wp.tile([49, BL, 4], f32, tag="y")
                nc.vector.tensor_sub(y[:], v4[:], mu[:].unsqueeze(2).broadcast_to([49, BL, 4]))
                nc.vector.tensor_mul(y[:], y[:], rstd[:].unsqueeze(2).broadcast_to([49, BL, 4]))
                nc.vector.tensor_mul(y[:], y[:], lng8.rearrange("p (b k) -> p b k", k=4))
                nc.vector.tensor_add(y[:], y[:], lnb8.rearrange("p (b k) -> p b k", k=4))
                z = wp.tile([49, BL, 4], f32, tag="z")
                nc.scalar.activation(z[:], y[:], Act.Exp, scale=1.0 / TEMP)
                ps_zs = pp_m.tile([1, BL * 4], f32, tag="psm")
                nc.tensor.matmul(ps_zs[:], ones49, z[:].rearrange("p b k -> p (b k)"),
                                 start=True, stop=True)
                zrec = wp.tile([1, BL * 4], f32, tag="zrec")
                nc.vector.reciprocal(zrec[:], ps_zs[:])
                ps_rb = pp_m.tile([49, BL * 4], f32, tag="psm")
                nc.tensor.matmul(ps_rb[:], ones1x[:, 0:49], zrec[:], start=True, stop=True)
                attn = wp.tile([49, BL, 4], f32, tag="attn")
                nc.vector.tensor_mul(attn[:], z[:], ps_rb[:].rearrange("p (b k) -> p b k", k=4))
                junk = wp.tile([49, 4], f32, tag="junk")
                asums = wp.tile([49, BL * 4], f32, tag="asums")
                for b in range(BL):
                    for s in range(4):
                        nc.vector.scalar_tensor_tensor(
                            junk[:], attn[:, b], 1.0, m4(s), op0=Alu.mult, op1=Alu.mult,
                            accum_out=asums[:, b * 4 + s:b * 4 + s + 1])
                ps_aw = pp_m.tile([1, BL * 4], f32, tag="psm")
                nc.tensor.matmul(ps_aw[:], ones49, asums[:], start=True, stop=True)
                aw_sb = wp.tile([1, BL * 4], f32, tag="aw_sb")
                nc.vector.tensor_copy(aw_sb[:], ps_aw[:])
                if debug_out:
                    nc.gpsimd.dma_start(out=dbg_v4[:], in_=v4[:])
                    nc.gpsimd.dma_start(out=dbg_aw[:], in_=aw_sb[:])
                ps_awb = pp_m.tile([64, BL * 4], f32, tag="psm")
                nc.tensor.matmul(ps_awb[:], ones1x, aw_sb[:], start=True, stop=True)
                awb = wp.tile([64, BL * 4], f32, tag="awb")
                nc.vector.tensor_copy(awb[:], ps_awb[:])

                # ================= per-sample: roi head, gather, interp =================
                for b in range(BL):
                    ctx9 = ctx9_l[b]
                    ps_r = pp_m.tile([16, 7, 7], f32, tag="psm")
                    for d in range(9):
                        dy, dx = d // 3, d % 3
                        nc.tensor.matmul(ps_r[:], roi1l(d), ctx9[:, dy:dy + 7, dx:dx + 7],
                                         start=(d == 0), stop=(d == 8))
                    r_sb = wp.tile([16, 7, 7], f32, tag="r_sb")
                    rsum = wp.tile([16, 1], f32, tag="rsum")
                    nc.scalar.activation(r_sb[:], ps_r[:], Act.Relu, accum_out=rsum[:])
                    ps_bb = pp_m.tile([4, 1], f32, tag="psm")
                    nc.tensor.matmul(ps_bb[:], w2l, rsum[:], start=True, stop=True)
                    bb_sb = wp.tile([4, 1], f32, tag="bb_sb")
                    nc.scalar.activation(bb_sb[:], ps_bb[:], Act.Identity, bias=b224, scale=224.0)
                    ps_bx = pp_m.tile([1, 4], f32, tag="psm")
                    nc.tensor.transpose(ps_bx[:], bb_sb[:], id7[0:4, 0:4])
                    bx = wp.tile([1, 4], f32, tag="bx")
                    nc.vector.tensor_copy(bx[:], ps_bx[:])

                    # box math on partition 0; layout [1, 14]
                    d2 = wp.tile([1, 2], f32, tag="d2")
                    nc.vector.tensor_sub(d2[:], bx[:, 2:4], bx[:, 0:2])
                    nc.vector.tensor_scalar_max(d2[:], d2[:], 1.0)
                    nc.vector.tensor_scalar_mul(d2[:], d2[:], 1.0 / 7.0)
                    cs = wp.tile([1, 2, 14], f32, tag="cs")        # row 0: xs, row 1: ys
                    for ax in range(2):
                        nc.vector.scalar_tensor_tensor(
                            cs[:, ax], offv, d2[:, ax:ax + 1], bx[:, ax:ax + 1].broadcast_to([1, 14]),
                            op0=Alu.mult, op1=Alu.add)
                    va = wp.tile([1, 2, 14], f32, tag="va")
                    vb = wp.tile([1, 2, 14], f32, tag="vb")
                    nc.vector.tensor_scalar(va[:], cs[:], -1.0, None, op0=Alu.is_ge)
                    nc.vector.tensor_scalar(vb[:], cs[:], 224.0, None, op0=Alu.is_le)
                    nc.vector.tensor_mul(va[:], va[:], vb[:])      # validity
                    # clamp, floor via trunc-convert, pair base c0c = min(floor, 222);
                    # fr relative to c0c makes the 223-edge fall out naturally
                    cc = wp.tile([1, 2, 14], f32, tag="cc")
                    nc.vector.tensor_scalar(cc[:], cs[:], 0.0, 223.0, op0=Alu.max, op1=Alu.min)
                    # floor robust to the convert's rounding mode (HW rounds
                    # half-even, CoreSim truncates): floor = conv(cc) - (conv > cc)
                    iraw = wp.tile([1, 2, 14], i32, tag="iraw")
                    nc.vector.tensor_copy(iraw[:], cc[:])
                    c0 = wp.tile([1, 2, 14], f32, tag="c0")
                    nc.vector.tensor_copy(c0[:], iraw[:])
                    cgt = wp.tile([1, 2, 14], f32, tag="cgt")
                    nc.vector.tensor_tensor(cgt[:], c0[:], cc[:], op=Alu.is_gt)
                    nc.vector.tensor_sub(c0[:], c0[:], cgt[:])
                    nc.vector.tensor_scalar_min(c0[:], c0[:], 222.0)
                    fr = wp.tile([1, 2, 14], f32, tag="fr")
                    nc.vector.tensor_sub(fr[:], cc[:], c0[:])
                    # weights wA = (1 - fr) * va ; wB = fr * va
                    # layout (axis, j, a): per-j (wA, wB) adjacent
                    wb_sb = wp.tile([1, 56], f32, tag="wb_sb")
                    tw = wp.tile([1, 2, 14], f32, tag="tw")
                    nc.vector.tensor_scalar(tw[:], fr[:], -1.0, 1.0, op0=Alu.mult, op1=Alu.add)
                    wbv = wb_sb[:].rearrange("p (t j a) -> p t j a", t=2, a=2)
                    for ax in range(2):                            # t0 = x-axis, t1 = y-axis
                        nc.vector.tensor_mul(wbv[:, ax, :, 0], tw[:, ax], va[:, ax])
                    for ax in range(2):
                        nc.vector.tensor_mul(wbv[:, ax, :, 1], fr[:, ax], va[:, ax])
                    # integer offsets: x cols, y row starts (elements)
                    ioff = wp.tile([1, 2, 14], f32, tag="ioff")
                    nc.vector.tensor_copy(ioff[:, 0], c0[:, 0])
                    nc.vector.tensor_scalar_mul(ioff[:, 1], c0[:, 1], 224.0)
                    ioffi = wp.tile([1, 2, 14], i32, tag="ioffi")
                    nc.vector.tensor_copy(ioffi[:], ioff[:])
                    if debug_out:
                        nc.gpsimd.dma_start(out=dbg_bx[b], in_=bx[:])
                        nc.gpsimd.dma_start(out=dbg_io[b], in_=ioffi[:])
                        nc.gpsimd.dma_start(out=dbg_wb[b], in_=wb_sb[:])

                    ps_wb = pp_m.tile([64, 56], f32, tag="psm")
                    nc.tensor.matmul(ps_wb[:], ones1x, wb_sb[:], start=True, stop=True)
                    wball = wp.tile([64, 2, 14, 2], f32, tag="wball")
                    nc.vector.tensor_copy(wball[:], ps_wb[:].rearrange("p (t j a) -> p t j a", t=2, a=2))

                    # ---- gather 14 row pairs ----
                    # spread the dynamic-offset work across engines (register files
                    # hold ~16 dyn values each): b0 -> SP gathers + DVE combines,
                    # b1 -> Act gathers + Pool combines
                    g2 = gp.tile([64, 14, 2, 224], f32, tag=f"g2{b}", name=f"g2{b}")
                    import concourse.mybir as _mb
                    dma_et, dma_eng = ((_mb.EngineType.SP, nc.sync) if b == 0
                                       else (_mb.EngineType.Activation, nc.scalar))
                    vec_et, vec_eng = ((_mb.EngineType.DVE, nc.vector) if b == 0
                                       else (_mb.EngineType.Pool, nc.gpsimd))
                    yvals = [nc.values_load(ioffi[:, 1, i:i + 1], engines=[dma_et],
                                            min_val=0, max_val=49728, skip_runtime_bounds_check=True)
                             for i in range(14)]
                    for i in range(14):
                        dma_eng.dma_start(out=g2[:, i], in_=xs_flat[b, :, ds(yvals[i], 448)]
                                          .rearrange("c (r w) -> c r w", r=2))
                    xvals = [nc.values_load(ioffi[:, 0, j:j + 1], engines=[vec_et],
                                            min_val=0, max_val=222, skip_runtime_bounds_check=True)
                             for j in range(14)]
                    # ---- col combine: pair-read [xv, xv+1] * (wA, wB), reduce ----
                    zc4 = wp.tile([64, 14, 2, 14], f32, tag="zc4")
                    prodc = wp.tile([64, 14, 2, 2], f32, tag="prodc")
                    for j in range(14):
                        vec_eng.tensor_tensor(
                            prodc[:], g2[:, :, :, ds(xvals[j], 2)],
                            wball[:, 0, j].unsqueeze(1).unsqueeze(1).broadcast_to([64, 14, 2, 2]),
                            op=Alu.mult)
                        vec_eng.tensor_tensor(zc4[:, :, :, j], prodc[:, :, :, 0],
                                              prodc[:, :, :, 1], op=Alu.add)
                    # ---- row combine ----
                    z14 = wp.tile([64, 14, 14], f32, tag="z14")
                    tmpr = wp.tile([64, 14], f32, tag="tmpr")
                    for i in range(14):
                        nc.vector.tensor_scalar(tmpr[:], zc4[:, i, 1, :], wball[:, 1, i, 1:2],
                                                None, op0=Alu.mult)
                        nc.vector.scalar_tensor_tensor(z14[:, i], zc4[:, i, 0, :],
                                                       wball[:, 1, i, 0:1], tmpr[:],
                                                       op0=Alu.mult, op1=Alu.add)
                    # ---- 2x2 subsample mean ----
                    z14v = z14[:].rearrange("p (i a) (j e) -> p i a j e", a=2, e=2)
                    t1 = wp.tile([64, 7, 7], f32, tag="t1")
                    t2 = wp.tile([64, 7, 7], f32, tag="t2")
                    nc.vector.tensor_add(t1[:], z14v[:, :, 0, :, 0], z14v[:, :, 0, :, 1])
                    nc.vector.tensor_add(t2[:], z14v[:, :, 1, :, 0], z14v[:, :, 1, :, 1])
                    nc.vector.tensor_add(t1[:], t1[:], t2[:])
                    zoomed = wp.tile([64, 9, 9], f32, tag="zoomed")
                    nc.gpsimd.memset(zoomed[:], 0.0)
                    nc.vector.tensor_scalar_mul(zoomed[:, 1:8, 1:8], t1[:], 0.25)
                    zoomed_l.append(zoomed)
                    if debug_out:
                        nc.gpsimd.dma_start(out=dbg_zm[b], in_=zoomed[:])
                        nc.gpsimd.dma_start(out=dbg_g2[b], in_=g2[:, 0].rearrange("c r w -> c (r w)"))

                # ================= final conv + combine =================
                for b in range(BL):
                    z9 = zoomed_l[b]
                    ps_fc = [pp_c.tile([64, 7, 7], f32, tag=f"ps_fc{k}", name=f"ps_fc{k}")
                             for k in range(4)]
                    for k in range(4):
                        for d in range(9):
                            dy, dx = d // 3, d % 3
                            nc.tensor.matmul(ps_fc[k][:], wl(k, d), z9[:, dy:dy + 7, dx:dx + 7],
                                             start=(d == 0), stop=(d == 8))
                    acc = wp.tile([64, 7, 7], f32, tag="acc")
                    nc.vector.tensor_scalar(acc[:], ps_fc[0][:], awb[:, b * 4:b * 4 + 1], None,
                                            op0=Alu.mult)
                    for k in range(1, 4):
                        nc.vector.scalar_tensor_tensor(acc[:], ps_fc[k][:],
                                                       awb[:, b * 4 + k:b * 4 + k + 1], acc[:],
                                                       op0=Alu.mult, op1=Alu.add)
                    aggb = wp.tile([64, 1], f32, tag="aggb")
                    nc.vector.tensor_scalar(aggb[:], biasT[:, 0:1], awb[:, b * 4:b * 4 + 1], None,
                                            op0=Alu.mult)
                    for k in range(1, 4):
                        nc.vector.scalar_tensor_tensor(aggb[:], biasT[:, k:k + 1],
                                                       awb[:, b * 4 + k:b * 4 + k + 1], aggb[:],
                                                       op0=Alu.mult, op1=Alu.add)
                    out_sb = wp.tile([64, 7, 7], f32, tag="out_sb")
                    nc.vector.tensor_scalar(out_sb[:], acc[:], aggb[:], None, op0=Alu.add)
                    nc.sync.dma_start(out=out_h[b], in_=out_sb[:])

    nc.compile()
    return nc


def get_nc():
    if "nc" not in _CACHED:
        _CACHED["nc"] = build_nc()
    return _CACHED["nc"]


# --------------------------------------------------------------------------
# entry point
# --------------------------------------------------------------------------
def kernel(**inputs):
    from concourse.bass_utils import run_bass_kernel_spmd

    x = np.ascontiguousarray(np.asarray(inputs["x"], np.float32))
    consts = make_consts(inputs)
    nc = get_nc()
    in_maps = [dict(xs=np.ascontiguousarray(x[c * BL:(c + 1) * BL]), **consts)
               for c in range(NCORES)]
    import ml_dtypes
    xb = x.transpose(0, 2, 1, 3).astype(ml_dtypes.bfloat16 if BF16_POOL else np.float32)
    for c in range(NCORES):
        in_maps[c]["xb"] = np.ascontiguousarray(xb[c * BL:(c + 1) * BL])
    res = run_bass_kernel_spmd(nc, in_maps, list(range(NCORES)))
    return np.concatenate([m["out"] for m in res.results], axis=0)



# revision 7
# speedup vs baseline: 2.2471x; 2.2471x over previous
"""Trainium2 Bass kernel for nn_CNN2D_37873021616665 (dense_cnn).

Data-parallel over batch: 16 samples -> 8 cores x 2 samples. Per core:

  pool stream   x shard as bf16 [h, c, w]; 8 chunk loads split over both
                HWDGE rings with sample 0's chunks first on each, so sample
                0's context is ready half-way through the stream and its
                whole tail (heads, gathers, bilinear combine, final conv)
                hides under sample 1's loads. 32x32 mean-pool on the PE:
                h contracted on 112 partitions x 2 groups, w binned by PSUM
                accumulation over 32 column offsets.
  heads         attention 1x1+3x3 conv and ROI bbox head per sample as soon
                as its context lands; LN(4)/softmax(49)/aw join runs inside
                sample 1's block. rstd = exp(-0.5*ln(var+eps)) so every ACT
                func (Ln/Exp) lives in one table set -> single table load
                (pinned by a t=0 Ln warmup); relus run as DVE max(x,0).
  roi_align     14 row-pair gathers per sample from a bf16 [c, h*w] copy:
                sample 0 on the ACT ring / sample 1 on the SP ring (each
                ring is free by the time its gathers are ready). Bilinear
                combine with clamp/validity folded into weights; sample 0
                combines on Pool, sample 1 on DVE (dyn-register budget:
                ACT=b0 rows, SP=b1 rows, Pool=b0 cols, DVE=b1 cols).
  final conv    by linearity: conv with the 4 base kernels packed as 2
                [64,128] kernel-pair stationaries (18 matmuls/sample),
                combined with the attention-derived scalars aw.
"""
import numpy as np

B = 16
NCORES = 8
BL = B // NCORES          # samples per core
C = 64
H = W = 224
HW = H * W
POOL = 7
NK = 4
TEMP = 5.0

_CACHED = {}


# --------------------------------------------------------------------------
# host-side constant packing (pure layout; replicated to all cores)
# --------------------------------------------------------------------------
def make_consts(inputs):
    import ml_dtypes
    f32 = np.float32
    fc1_w = np.asarray(inputs["fc1_w"], f32)
    fc2_w = np.asarray(inputs["fc2_w"], f32)
    fc2_b = np.asarray(inputs["fc2_b"], f32)
    ln_g = np.asarray(inputs["ln_g"], f32)
    ln_b = np.asarray(inputs["ln_b"], f32)
    roi_w1 = np.asarray(inputs["roi_w1"], f32)
    roi_w2 = np.asarray(inputs["roi_w2"], f32)
    roi_b2 = np.asarray(inputs["roi_b2"], f32)
    weight = np.asarray(inputs["weight"], f32)
    bias = np.asarray(inputs["bias"], f32)

    # pool matrices: h = g*112 + p contracts on partitions, bin = h // 32
    k112a = np.zeros((112, 7), f32)
    k112b = np.zeros((112, 7), f32)
    for p in range(112):
        k112a[p, p // 32] = 1.0 / 1024.0
        k112b[p, (112 + p) // 32] = 1.0 / 1024.0
    k112a = k112a.astype(ml_dtypes.bfloat16)   # 1/1024 exact in bf16
    k112b = k112b.astype(ml_dtypes.bfloat16)

    k64 = np.zeros((64, 2468), f32)
    k64[:, 0:16] = fc1_w[:, :, 0, 0].T
    k64[:, 16:160] = roi_w1.transpose(1, 2, 3, 0).reshape(64, 144)
    # final-conv stationaries: kernel-pairs packed into [64, 128] blocks,
    # block(pair, d) at 160 + (pair*9 + d)*128
    wt = weight.transpose(2, 0, 3, 4, 1)        # [c, k, dy, dx, o]
    for pr in range(2):
        for d in range(9):
            dy, dx = d // 3, d % 3
            col = 160 + (pr * 9 + d) * 128
            k64[:, col:col + 64] = wt[:, 2 * pr, dy, dx, :]
            k64[:, col + 64:col + 128] = wt[:, 2 * pr + 1, dy, dx, :]
    k64[:, 2464:2468] = bias.T

    k49 = np.zeros((49, 33), f32)
    k49[:, 0:8] = np.tile(ln_g, (49, 2))
    k49[:, 8:16] = np.tile(ln_b, (49, 2))
    t = np.arange(196).reshape(49, 4)
    for s in range(4):
        k49[:, 16 + s * 4:16 + (s + 1) * 4] = ((t >= 49 * s) & (t < 49 * (s + 1)))
    k49[:, 32] = 1.0

    k16 = np.zeros((16, 40), f32)
    k16[:, 0:36] = fc2_w.transpose(1, 2, 3, 0).reshape(16, 36)
    k16[:, 36:40] = (roi_w2[:4, :, 0, 0] / 49.0).T

    k7 = np.eye(7, dtype=f32)

    k4 = np.zeros((4, 2), f32)
    k4[:, 0] = fc2_b
    k4[:, 1] = roi_b2[:4] * 224.0

    k1 = np.zeros((1, 78), f32)
    off = (np.arange(7, dtype=f32)[:, None] + (np.arange(2, dtype=f32)[None, :] + 0.5) / 2)
    k1[0, 0:14] = off.reshape(-1)
    k1[0, 14:78] = 1.0

    return {"k112a": k112a, "k112b": k112b, "k64": k64, "k49": k49,
            "k16": k16, "k7": k7, "k4": k4, "k1": k1}


def make_in_maps(inputs):
    """Full inputs -> per-core input maps (host-side layout staging only)."""
    import ml_dtypes
    x = np.ascontiguousarray(np.asarray(inputs["x"], np.float32))
    consts = make_consts(inputs)
    bf16 = ml_dtypes.bfloat16
    xb = x.transpose(0, 2, 1, 3).astype(bf16)       # [B, h, c, w] pool stream
    xg = x.astype(bf16).reshape(B, C, HW)            # [B, c, h*w] gather source
    maps = []
    for c in range(NCORES):
        maps.append(dict(
            xb=np.ascontiguousarray(xb[c * BL:(c + 1) * BL]),
            xg=np.ascontiguousarray(xg[c * BL:(c + 1) * BL]),
            **consts))
    return maps


# --------------------------------------------------------------------------
# device program
# --------------------------------------------------------------------------
def build_nc():
    import concourse.bass as bass
    import concourse.bacc as bacc
    import concourse.mybir as mybir
    import concourse.tile as tile
    from concourse.bass import ds

    f32 = mybir.dt.float32
    i32 = mybir.dt.int32
    bf16 = mybir.dt.bfloat16
    Alu = mybir.AluOpType
    Act = mybir.ActivationFunctionType
    ET = mybir.EngineType

    nc = bacc.Bacc("TRN2", target_bir_lowering=False)

    xb_h = nc.dram_tensor("xb", [BL, H, C, W], bf16, kind="ExternalInput")
    xg_h = nc.dram_tensor("xg", [BL, C, HW], bf16, kind="ExternalInput")
    k112a_h = nc.dram_tensor("k112a", [112, 7], bf16, kind="ExternalInput")
    k112b_h = nc.dram_tensor("k112b", [112, 7], bf16, kind="ExternalInput")
    k64_h = nc.dram_tensor("k64", [64, 2468], f32, kind="ExternalInput")
    k49_h = nc.dram_tensor("k49", [49, 33], f32, kind="ExternalInput")
    k16_h = nc.dram_tensor("k16", [16, 40], f32, kind="ExternalInput")
    k7_h = nc.dram_tensor("k7", [7, 7], f32, kind="ExternalInput")
    k4_h = nc.dram_tensor("k4", [4, 2], f32, kind="ExternalInput")
    k1_h = nc.dram_tensor("k1", [1, 78], f32, kind="ExternalInput")
    out_h = nc.dram_tensor("out", [BL, C, POOL, POOL], f32, kind="ExternalOutput")

    from contextlib import ExitStack

    with tile.TileContext(nc) as tc, ExitStack() as est:
        cp = est.enter_context(tc.tile_pool(name="const", bufs=1))
        xp = est.enter_context(tc.tile_pool(name="xtiles", bufs=1))
        wp = est.enter_context(tc.tile_pool(name="work", bufs=2))
        gp = est.enter_context(tc.tile_pool(name="gather", bufs=1))
        pp_acc = est.enter_context(tc.tile_pool(name="ps_acc", bufs=2, space="PSUM"))
        pp_m1 = est.enter_context(tc.tile_pool(name="ps_m1", bufs=2, space="PSUM"))
        pp_m2 = est.enter_context(tc.tile_pool(name="ps_m2", bufs=2, space="PSUM"))
        pp_c = est.enter_context(tc.tile_pool(name="ps_conv", bufs=1, space="PSUM"))
        dp = est.enter_context(tc.tile_pool(name="dscratch", bufs=1, space="DRAM"))

        # ---- pool-matrix consts first on ACT ring (needed by first matmul) ----
        kc = {}
        for name, hdl, shp, dt in (("k112a", k112a_h, [112, 7], bf16),
                                   ("k112b", k112b_h, [112, 7], bf16)):
            t = cp.tile(shp, dt, tag=name)
            nc.scalar.dma_start(out=t[:], in_=hdl[:])
            kc[name] = t

        # ---- pool loads: SP ring gets b0 cg0/1 then b1 cg0/1;
        #      ACT ring gets b0 cg2/3, (consts), b1 cg2/3 ----
        def load_chunk(eng, b, cg):
            t = xp.tile([112, 2, 16, W], bf16, tag=f"x{b}{cg}", name=f"x{b}{cg}")
            eng.dma_start(out=t[:],
                          in_=xb_h[b, :, cg * 16:(cg + 1) * 16, :]
                          .rearrange("(g p) c w -> p g c w", g=2))
            return t

        xt = {}
        xt[(0, 0)] = load_chunk(nc.sync, 0, 0)
        xt[(0, 1)] = load_chunk(nc.sync, 0, 1)
        xt[(1, 0)] = load_chunk(nc.sync, 1, 0)
        xt[(1, 1)] = load_chunk(nc.sync, 1, 1)
        xt[(0, 2)] = load_chunk(nc.scalar, 0, 2)
        xt[(0, 3)] = load_chunk(nc.scalar, 0, 3)

        # ---- remaining consts on ACT ring ----
        for name, hdl, shp in (("k64", k64_h, [64, 2468]), ("k49", k49_h, [49, 33]),
                               ("k16", k16_h, [16, 40]), ("k7", k7_h, [7, 7]),
                               ("k4", k4_h, [4, 2]), ("k1", k1_h, [1, 78])):
            t = cp.tile(shp, f32, tag=name)
            nc.scalar.dma_start(out=t[:], in_=hdl[:])
            kc[name] = t

        xt[(1, 2)] = load_chunk(nc.scalar, 1, 2)
        xt[(1, 3)] = load_chunk(nc.scalar, 1, 3)

        fc1l = kc["k64"][:, 0:16]
        roi1l = lambda d: kc["k64"][:, 16 + d * 16:16 + (d + 1) * 16]
        wpair = lambda pr, d: kc["k64"][:, 160 + (pr * 9 + d) * 128:160 + (pr * 9 + d + 1) * 128]
        biasT = kc["k64"][:, 2464:2468]
        lng8 = kc["k49"][:, 0:8]
        lnb8 = kc["k49"][:, 8:16]
        m4 = lambda s: kc["k49"][:, 16 + s * 4:16 + (s + 1) * 4]
        ones49 = kc["k49"][:, 32:33]
        fc2l = lambda d: kc["k16"][:, d * 4:(d + 1) * 4]
        w2l = kc["k16"][:, 36:40]
        id7 = kc["k7"][:]
        fc2b = kc["k4"][:, 0:1]
        b224 = kc["k4"][:, 1:2]
        offv = kc["k1"][:, 0:14]
        ones1x = kc["k1"][:, 14:78]          # [1, 64] of ones

        # ---- early zero-pad memsets (Pool) + ACT table warmup (Ln -> the
        #      natural_log_exp set, the only set every ACT func here uses) ----
        warm = wp.tile([1, 2], f32, tag="warm")
        nc.gpsimd.memset(warm[:], 1.0)
        warm2 = wp.tile([1, 2], f32, tag="warm2")
        nc.scalar.activation(warm2[:], warm[:], Act.Ln)
        ctx9_l, a9_l, zoomed_l = [], [], []
        for b in range(BL):
            ctx9 = wp.tile([64, 9, 9], f32, tag=f"ctx9{b}", name=f"ctx9{b}")
            nc.gpsimd.memset(ctx9[:], 0.0)
            ctx9_l.append(ctx9)
            a9 = wp.tile([16, 9, 9], f32, tag=f"a9{b}", name=f"a9{b}")
            nc.gpsimd.memset(a9[:], 0.0)
            a9_l.append(a9)
            zoomed = wp.tile([64, 9, 9], f32, tag=f"zoomed{b}", name=f"zoomed{b}")
            nc.gpsimd.memset(zoomed[:], 0.0)
            zoomed_l.append(zoomed)

        attflat = dp.tile([BL, 196], f32)
        awb = wp.tile([64, BL * 4], f32, tag="awb")

        # ================= per-sample: pool -> heads -> gathers -> combine ====
        for b in range(BL):
            # ---- pooling matmuls ----
            ps_pool = pp_acc.tile([7, 4, 16, 7], f32, tag="poolacc")
            for cg in range(4):
                t4 = xt[(b, cg)][:].rearrange("p g c (j t) -> p g c j t", t=32)
                for g, kq in ((0, kc["k112a"]), (1, kc["k112b"])):
                    for s in range(32):
                        nc.tensor.matmul(ps_pool[:, cg], kq[:], t4[:, g, :, :, s],
                                         start=(g == 0 and s == 0),
                                         stop=(g == 1 and s == 31))
            ctx_sb = wp.tile([7, 4, 16, 7], f32, tag="ctx_sb")
            nc.vector.tensor_copy(ctx_sb[:], ps_pool[:])

            # ---- transpose context to [c, i, j] ----
            ps_ctxT = pp_m1.tile([64, 7, 7], f32, tag="psm")
            for j in range(7):
                nc.tensor.transpose(ps_ctxT[:, :, j], ctx_sb[:, :, :, j], id7)
            ctxT = wp.tile([64, 7, 7], f32, tag="ctxT")
            nc.vector.tensor_copy(ctxT[:], ps_ctxT[:])
            ctx9 = ctx9_l[b]
            nc.vector.tensor_copy(ctx9[:, 1:8, 1:8], ctxT[:])

            # ---- roi bbox head ----
            ps_r = pp_m1.tile([16, 7, 7], f32, tag="psm")
            for d in range(9):
                dy, dx = d // 3, d % 3
                nc.tensor.matmul(ps_r[:], roi1l(d), ctx9[:, dy:dy + 7, dx:dx + 7],
                                 start=(d == 0), stop=(d == 8))
            r_sb = wp.tile([16, 7, 7], f32, tag="r_sb")
            rsum = wp.tile([16, 1], f32, tag="rsum")
            nc.vector.tensor_scalar_max(r_sb[:], ps_r[:], 0.0)
            nc.vector.tensor_reduce(rsum[:], r_sb[:].rearrange("p i j -> p (i j)"),
                                    op=Alu.add, axis=mybir.AxisListType.X)
            ps_bb = pp_m1.tile([4, 1], f32, tag="psm")
            nc.tensor.matmul(ps_bb[:], w2l, rsum[:], start=True, stop=True)
            bb_sb = wp.tile([4, 1], f32, tag="bb_sb")
            nc.vector.tensor_scalar(bb_sb[:], ps_bb[:], 224.0, b224, op0=Alu.mult, op1=Alu.add)
            ps_bx = pp_m1.tile([1, 4], f32, tag="psm")
            nc.tensor.transpose(ps_bx[:], bb_sb[:], id7[0:4, 0:4])
            bx = wp.tile([1, 4], f32, tag="bx")
            nc.vector.tensor_copy(bx[:], ps_bx[:])

            # ---- box math on partition 0; layout [1, 14] ----
            d2 = wp.tile([1, 2], f32, tag="d2")
            nc.vector.tensor_sub(d2[:], bx[:, 2:4], bx[:, 0:2])
            nc.vector.tensor_scalar_max(d2[:], d2[:], 1.0)
            nc.vector.tensor_scalar_mul(d2[:], d2[:], 1.0 / 7.0)
            cs = wp.tile([1, 2, 14], f32, tag="cs")        # row 0: xs, row 1: ys
            for ax in range(2):
                nc.vector.scalar_tensor_tensor(
                    cs[:, ax], offv, d2[:, ax:ax + 1], bx[:, ax:ax + 1].broadcast_to([1, 14]),
                    op0=Alu.mult, op1=Alu.add)
            va = wp.tile([1, 2, 14], f32, tag="va")
            vb = wp.tile([1, 2, 14], f32, tag="vb")
            nc.vector.tensor_scalar(va[:], cs[:], -1.0, None, op0=Alu.is_ge)
            nc.vector.tensor_scalar(vb[:], cs[:], 224.0, None, op0=Alu.is_le)
            nc.vector.tensor_mul(va[:], va[:], vb[:])      # validity
            # clamp; floor robust to convert rounding mode; pair base <= 222
            cc = wp.tile([1, 2, 14], f32, tag="cc")
            nc.vector.tensor_scalar(cc[:], cs[:], 0.0, 223.0, op0=Alu.max, op1=Alu.min)
            iraw = wp.tile([1, 2, 14], i32, tag="iraw")
            nc.vector.tensor_copy(iraw[:], cc[:])
            c0 = wp.tile([1, 2, 14], f32, tag="c0")
            nc.vector.tensor_copy(c0[:], iraw[:])
            cgt = wp.tile([1, 2, 14], f32, tag="cgt")
            nc.vector.tensor_tensor(cgt[:], c0[:], cc[:], op=Alu.is_gt)
            nc.vector.tensor_sub(c0[:], c0[:], cgt[:])
            nc.vector.tensor_scalar_min(c0[:], c0[:], 222.0)
            fr = wp.tile([1, 2, 14], f32, tag="fr")
            nc.vector.tensor_sub(fr[:], cc[:], c0[:])
            # weights wA = (1 - fr) * va ; wB = fr * va, layout (axis, j, a)
            wb_sb = wp.tile([1, 56], f32, tag="wb_sb")
            tw = wp.tile([1, 2, 14], f32, tag="tw")
            nc.vector.tensor_scalar(tw[:], fr[:], -1.0, 1.0, op0=Alu.mult, op1=Alu.add)
            wbv = wb_sb[:].rearrange("p (t j a) -> p t j a", t=2, a=2)
            for ax in range(2):                            # t0 = x-axis, t1 = y-axis
                nc.vector.tensor_mul(wbv[:, ax, :, 0], tw[:, ax], va[:, ax])
            for ax in range(2):
                nc.vector.tensor_mul(wbv[:, ax, :, 1], fr[:, ax], va[:, ax])
            # fold the 2x2 subsample mean into the bilinear weights (0.5/axis)
            nc.vector.tensor_scalar_mul(wb_sb[:], wb_sb[:], 0.5)
            # integer offsets: x cols, y row starts (elements)
            ioff = wp.tile([1, 2, 14], f32, tag="ioff")
            nc.vector.tensor_copy(ioff[:, 0], c0[:, 0])
            nc.vector.tensor_scalar_mul(ioff[:, 1], c0[:, 1], 224.0)
            ioffi = wp.tile([1, 2, 14], i32, tag="ioffi")
            nc.vector.tensor_copy(ioffi[:], ioff[:])

            ps_wb = pp_m1.tile([64, 56], f32, tag="psm")
            nc.tensor.matmul(ps_wb[:], ones1x, wb_sb[:], start=True, stop=True)
            wball = wp.tile([64, 2, 14, 2], f32, tag="wball")
            nc.vector.tensor_copy(wball[:], ps_wb[:].rearrange("p (t j a) -> p t j a", t=2, a=2))

            # ---- attention head ----
            ps_a = pp_m1.tile([16, 7, 7], f32, tag="psm")
            nc.tensor.matmul(ps_a[:], fc1l, ctxT[:], start=True, stop=True)
            a_sb = wp.tile([16, 7, 7], f32, tag="a_sb")
            nc.vector.tensor_scalar_max(a_sb[:], ps_a[:], 0.0)
            a9 = a9_l[b]
            nc.vector.tensor_copy(a9[:, 1:8, 1:8], a_sb[:])
            ps_att = pp_m1.tile([4, 7, 7], f32, tag="psm")
            for d in range(9):
                dy, dx = d // 3, d % 3
                nc.tensor.matmul(ps_att[:], fc2l(d), a9[:, dy:dy + 7, dx:dx + 7],
                                 start=(d == 0), stop=(d == 8))
            att_sb = wp.tile([4, 7, 7], f32, tag="att_sb")
            nc.vector.tensor_scalar(att_sb[:], ps_att[:], fc2b, None, op0=Alu.add)
            # attflat round-trips ride the SP ring (free after its pool loads)
            nc.sync.dma_start(
                out=attflat[b].rearrange("(k q) -> k q", k=4),
                in_=att_sb[:].rearrange("k i j -> k (i j)"))

            if b == BL - 1:
                # ============ regroup + LN + softmax + aw (both samples) ======
                v4 = wp.tile([49, BL, 4], f32, tag="v4")
                nc.sync.dma_start(out=v4[:], in_=attflat[:].rearrange("b (p k) -> p b k", k=4))
                s1 = wp.tile([49, BL], f32, tag="s1")
                nc.vector.tensor_reduce(s1[:], v4[:], op=Alu.add, axis=mybir.AxisListType.X)
                sq = wp.tile([49, BL, 4], f32, tag="sq")
                nc.vector.tensor_mul(sq[:], v4[:], v4[:])
                s2 = wp.tile([49, BL], f32, tag="s2")
                nc.vector.tensor_reduce(s2[:], sq[:], op=Alu.add, axis=mybir.AxisListType.X)
                mu = wp.tile([49, BL], f32, tag="mu")
                nc.vector.tensor_scalar_mul(mu[:], s1[:], 0.25)
                mu2 = wp.tile([49, BL], f32, tag="mu2")
                nc.vector.tensor_mul(mu2[:], mu[:], mu[:])
                var = wp.tile([49, BL], f32, tag="var")
                nc.vector.scalar_tensor_tensor(var[:], s2[:], 0.25, mu2[:],
                                               op0=Alu.mult, op1=Alu.subtract)
                nc.vector.tensor_scalar_add(var[:], var[:], 1e-5)
                # rstd = exp(-0.5 * ln(var)): stays on the exp/ln table set
                lnv = wp.tile([49, BL], f32, tag="lnv")
                nc.scalar.activation(lnv[:], var[:], Act.Ln)
                rstd = wp.tile([49, BL], f32, tag="rstd")
                nc.scalar.activation(rstd[:], lnv[:], Act.Exp, scale=-0.5)
                y = wp.tile([49, BL, 4], f32, tag="y")
                nc.vector.tensor_sub(y[:], v4[:], mu[:].unsqueeze(2).broadcast_to([49, BL, 4]))
                nc.vector.tensor_mul(y[:], y[:], rstd[:].unsqueeze(2).broadcast_to([49, BL, 4]))
                nc.vector.tensor_mul(y[:], y[:], lng8.rearrange("p (b k) -> p b k", k=4))
                nc.vector.tensor_add(y[:], y[:], lnb8.rearrange("p (b k) -> p b k", k=4))
                z = wp.tile([49, BL, 4], f32, tag="z")
                nc.scalar.activation(z[:], y[:], Act.Exp, scale=1.0 / TEMP)
                ps_zs = pp_m2.tile([1, BL * 4], f32, tag="psm2")
                nc.tensor.matmul(ps_zs[:], ones49, z[:].rearrange("p b k -> p (b k)"),
                                 start=True, stop=True)
                zrec = wp.tile([1, BL * 4], f32, tag="zrec")
                nc.vector.reciprocal(zrec[:], ps_zs[:])
                ps_rb = pp_m2.tile([49, BL * 4], f32, tag="psm2")
                nc.tensor.matmul(ps_rb[:], ones1x[:, 0:49], zrec[:], start=True, stop=True)
                attn = wp.tile([49, BL, 4], f32, tag="attn")
                nc.vector.tensor_mul(attn[:], z[:], ps_rb[:].rearrange("p (b k) -> p b k", k=4))
                junk = wp.tile([49, 4], f32, tag="junk")
                asums = wp.tile([49, BL * 4], f32, tag="asums")
                for bb in range(BL):
                    for s in range(4):
                        nc.vector.scalar_tensor_tensor(
                            junk[:], attn[:, bb], 1.0, m4(s), op0=Alu.mult, op1=Alu.mult,
                            accum_out=asums[:, bb * 4 + s:bb * 4 + s + 1])
                ps_aw = pp_m2.tile([1, BL * 4], f32, tag="psm2")
                nc.tensor.matmul(ps_aw[:], ones49, asums[:], start=True, stop=True)
                aw_sb = wp.tile([1, BL * 4], f32, tag="aw_sb")
                nc.vector.tensor_copy(aw_sb[:], ps_aw[:])
                ps_awb = pp_m2.tile([64, BL * 4], f32, tag="psm2")
                nc.tensor.matmul(ps_awb[:], ones1x, aw_sb[:], start=True, stop=True)
                nc.vector.tensor_copy(awb[:], ps_awb[:])

            # ---- gather 14 row pairs (bf16 source) ----
            # b=0: ACT ring + Pool combine; b=1: SP ring + DVE combine
            g2 = gp.tile([64, 14, 2, W], bf16, tag=f"g2{b}", name=f"g2{b}")
            dma_et, dma_eng = ((ET.Activation, nc.scalar) if b == 0
                               else (ET.SP, nc.sync))
            vec_et, vec_eng = ((ET.Pool, nc.gpsimd) if b == 0
                               else (ET.DVE, nc.vector))
            yvals = [nc.values_load(ioffi[:, 1, i:i + 1], engines=[dma_et],
                                    min_val=0, max_val=49728, skip_runtime_bounds_check=True)
                     for i in range(14)]
            for i in range(14):
                dma_eng.dma_start(out=g2[:, i], in_=xg_h[b, :, ds(yvals[i], 448)]
                                  .rearrange("c (r w) -> c r w", r=2))

            # ---- col combine ----
            xvals = [nc.values_load(ioffi[:, 0, j:j + 1], engines=[vec_et],
                                    min_val=0, max_val=222, skip_runtime_bounds_check=True)
                     for j in range(14)]
            zc4 = wp.tile([64, 14, 2, 14], f32, tag=f"zc4{b}", name=f"zc4{b}")
            prodc = wp.tile([64, 14, 2, 2], f32, tag="prodc")
            for j in range(14):
                vec_eng.tensor_tensor(
                    prodc[:], g2[:, :, :, ds(xvals[j], 2)],
                    wball[:, 0, j].unsqueeze(1).unsqueeze(1).broadcast_to([64, 14, 2, 2]),
                    op=Alu.mult)
                vec_eng.tensor_tensor(zc4[:, :, :, j], prodc[:, :, :, 0],
                                      prodc[:, :, :, 1], op=Alu.add)

            # ---- row combine (broadcast along j; Pool-safe tensor_tensor) ----
            z14 = wp.tile([64, 14, 14], f32, tag=f"z14{b}", name=f"z14{b}")
            rta = wp.tile([64, 14, 14], f32, tag="rta")
            rtb = wp.tile([64, 14, 14], f32, tag="rtb")
            vec_eng.tensor_tensor(
                rta[:], zc4[:, :, 0, :],
                wball[:, 1, :, 0].unsqueeze(2).broadcast_to([64, 14, 14]), op=Alu.mult)
            vec_eng.tensor_tensor(
                rtb[:], zc4[:, :, 1, :],
                wball[:, 1, :, 1].unsqueeze(2).broadcast_to([64, 14, 14]), op=Alu.mult)
            vec_eng.tensor_tensor(z14[:], rta[:], rtb[:], op=Alu.add)
            # ---- 2x2 subsample sum (mean folded into weights) ----
            z14v = z14[:].rearrange("p (i a) (j e) -> p i a j e", a=2, e=2)
            t1 = wp.tile([64, 7, 7], f32, tag="t1")
            t2 = wp.tile([64, 7, 7], f32, tag="t2")
            vec_eng.tensor_tensor(t1[:], z14v[:, :, 0, :, 0], z14v[:, :, 0, :, 1], op=Alu.add)
            vec_eng.tensor_tensor(t2[:], z14v[:, :, 1, :, 0], z14v[:, :, 1, :, 1], op=Alu.add)
            vec_eng.tensor_tensor(zoomed_l[b][:, 1:8, 1:8], t1[:], t2[:], op=Alu.add)

        # ================= final conv (kernel-pairs packed) + combine =========
        for b in range(BL):
            z9 = zoomed_l[b]
            ps_pr = [pp_c.tile([128, 7, 7], f32, tag=f"ps_pr{pr}", name=f"ps_pr{pr}")
                     for pr in range(2)]
            for pr in range(2):
                for d in range(9):
                    dy, dx = d // 3, d % 3
                    nc.tensor.matmul(ps_pr[pr][:], wpair(pr, d), z9[:, dy:dy + 7, dx:dx + 7],
                                     start=(d == 0), stop=(d == 8))
            acc = wp.tile([64, 7, 7], f32, tag="acc")
            nc.vector.tensor_scalar(acc[:], ps_pr[0][0:64], awb[:, b * 4:b * 4 + 1], None,
                                    op0=Alu.mult)
            for k in range(1, 4):
                nc.vector.scalar_tensor_tensor(acc[:], ps_pr[k // 2][(k % 2) * 64:(k % 2) * 64 + 64],
                                               awb[:, b * 4 + k:b * 4 + k + 1], acc[:],
                                               op0=Alu.mult, op1=Alu.add)
            aggb = wp.tile([64, 1], f32, tag="aggb")
            nc.vector.tensor_scalar(aggb[:], biasT[:, 0:1], awb[:, b * 4:b * 4 + 1], None,
                                    op0=Alu.mult)
            for k in range(1, 4):
                nc.vector.scalar_tensor_tensor(aggb[:], biasT[:, k:k + 1],
                                               awb[:, b * 4 + k:b * 4 + k + 1], aggb[:],
                                               op0=Alu.mult, op1=Alu.add)
            out_sb = wp.tile([64, 7, 7], f32, tag="out_sb")
            nc.vector.tensor_scalar(out_sb[:], acc[:], aggb[:], None, op0=Alu.add)
            nc.scalar.dma_start(out=out_h[b], in_=out_sb[:])

    nc.compile()
    return nc


def get_nc():
    if "nc" not in _CACHED:
        _CACHED["nc"] = build_nc()
    return _CACHED["nc"]


# --------------------------------------------------------------------------
# entry point
# --------------------------------------------------------------------------
def kernel(**inputs):
    from concourse.bass_utils import run_bass_kernel_spmd

    nc = get_nc()
    in_maps = make_in_maps(inputs)
    res = run_bass_kernel_spmd(nc, in_maps, list(range(NCORES)))
    return np.concatenate([m["out"] for m in res.results], axis=0)
